# revision 1
# baseline (speedup 1.0000x reference)
"""Trainium2 Bass kernel for the GraphicalBranch GNN message-passing problem.

Math being computed (verified equivalent to the reference):
  - Per-sample graphs are fully connected WITH self-loops over the nc2=28
    pair-nodes, so segment_sum(x[src], dst) == broadcast of the per-sample
    row-sum S[b] = sum_r x[b, r, :].
  - The final key-matching gather h[rows] commutes with the row-wise linear
    layer, so we only run the W_self matmul on the 10 gathered rows per
    sample instead of all 28:
        out[b*10+k] = relu(xg[b*10+k] @ W_self + (S[b] @ W_nbr) + b)
  - rows are computed on host from slicing_tensor/object_pairs (pure index
    arithmetic) exactly as the reference's LUT does.

Sharding: data-parallel over samples; each of the 8 cores gets 128 samples
(3584 x-rows, 1280 output rows). Weights replicated.

Per-core device program (matmul operands bf16, f32 PSUM accumulate):
  1. S = G^T @ x on TensorE. x streams in 4 chunks of 896 rows (= exactly
     32 samples), so the same tiny one-hot block G[j][p, s] =
     ((j*128+p)//28 == s), s in [0,32), works for every chunk; chunk ch
     accumulates into PSUM partitions [32ch, 32ch+32) via
     tile_position=(0, 32ch). W_self matmuls for output tiles 0-3 are
     interleaved between chunks to fill PE gaps (their PSUM groups stay
     open until A is ready).
  2. Transpose S via 4 PE transposes -> S^T tiles (bf16).
  3. A = S @ W_nbr + b via 4 accumulating matmuls plus a K=1 ones-matmul
     that adds b to every row of the PSUM accumulator.
  4. Per output tile: 4 matmuls xg @ W_self (lhsT = xgT slices) + one
     expansion matmul E_t @ A (E[r, s] = 1 iff s == r//10) closing the
     same PSUM accumulation group; ReLU on ScalarE; stores in pairs.

All DRAM inputs are host-prelaid so every load is a plain contiguous
[128, F] DMA (one descriptor per partition). Loads split across the two
HWDGE rings in consumption order: the sync ring carries the big tensors
(x0, xgT, ws, x1..x3 — a single FIFO so arrival order is deterministic
and the PE never head-of-line blocks), the scalar ring the small/tail
set (g, wn, id, b, eT).
"""

import numpy as np
import ml_dtypes

# ---- problem constants (hardcoded; kernel.py must be self-contained) ----
B = 1024          # samples
NOBJ = 8          # objects per sample
NC2 = 28          # pair-nodes per sample
MAXR = 10         # relations per sample
D = 512           # feature dim
NCORES = 8
BL = B // NCORES          # 128 samples per core
RL = BL * NC2             # 3584 x-rows per core
ML = BL * MAXR            # 1280 output rows per core
KT = D // 128             # 4 contraction tiles
MT = ML // 128            # 10 output row tiles per core
RT = RL // 128            # 28 x row-tiles per core
XCH = 4                   # x chunks (896 rows = 32 samples each)
RJ = RT // XCH            # 7 row-tiles per chunk
SW = BL // XCH            # 32 samples per chunk

BF16 = ml_dtypes.bfloat16

_compiled = None


def _build_bass():
    import concourse.bacc as bacc
    import concourse.bass as bass
    import concourse.mybir as mybir
    from concourse import tile

    f32 = mybir.dt.float32
    bf16 = mybir.dt.bfloat16

    nc = bacc.Bacc("TRN2", target_bir_lowering=False, debug=False,
                   num_devices=NCORES)

    # all inputs prelaid on host: partition-major, contiguous free dim
    x_d = nc.dram_tensor("x", [XCH, 128, RJ * D], bf16, kind="ExternalInput")
    g_d = nc.dram_tensor("g", [128, RJ * SW], bf16, kind="ExternalInput")
    xgT_d = nc.dram_tensor("xgT", [128, KT * ML], bf16, kind="ExternalInput")
    ws_d = nc.dram_tensor("ws", [128, KT * D], bf16, kind="ExternalInput")
    wn_d = nc.dram_tensor("wn", [128, KT * D], bf16, kind="ExternalInput")
    eT_d = nc.dram_tensor("eT", [128, ML], bf16, kind="ExternalInput")
    b_d = nc.dram_tensor("bias", [1, D], bf16, kind="ExternalInput")
    id_d = nc.dram_tensor("ident", [128, 128], bf16, kind="ExternalInput")
    out_d = nc.dram_tensor("out", [ML, D], f32, kind="ExternalOutput")

    with tile.TileContext(nc) as tc:
        with (
            tc.tile_pool(name="const", bufs=1) as cpool,
            tc.tile_pool(name="x", bufs=4) as xpool,
            tc.tile_pool(name="outp", bufs=3) as opool,
            tc.tile_pool(name="psum", bufs=4, space=bass.MemorySpace.PSUM) as ppool,
            tc.tile_pool(name="psumS", bufs=1, space=bass.MemorySpace.PSUM) as pspool,
            tc.tile_pool(name="psumT", bufs=2, space=bass.MemorySpace.PSUM) as ptpool,
            tc.tile_pool(name="psumA", bufs=1, space=bass.MemorySpace.PSUM) as papool,
        ):
            # ---- loads: sync ring carries x0, xgT, ws, x1..x3 in
            # ---- consumption order; scalar ring carries the small/tail set
            g_sb = cpool.tile([128, RJ, SW], bf16)
            nc.scalar.dma_start(g_sb[:], g_d.rearrange("p (j s) -> p j s", s=SW))
            wn_sb = cpool.tile([128, KT, D], bf16)
            nc.scalar.dma_start(wn_sb[:], wn_d.rearrange("p (t n) -> p t n", n=D))
            id_sb = cpool.tile([128, 128], bf16)
            nc.scalar.dma_start(id_sb[:], id_d[:, :])
            b_sb = cpool.tile([1, D], bf16)
            nc.scalar.dma_start(b_sb[:], b_d[:, :])
            eT_sb = cpool.tile([128, ML], bf16)
            nc.scalar.dma_start(eT_sb[:], eT_d[:, :])
            ones_sb = cpool.tile([1, 128], bf16)
            nc.gpsimd.memset(ones_sb[:], 1.0)

            # ---- S accumulation, interleaved with early W_self groups ----
            psS = pspool.tile([128, D], f32)
            main_ps = {}

            def open_main_group(t):
                ps = ppool.tile([128, D], f32, tag="ps")
                for kt in range(KT):
                    nc.tensor.matmul(
                        ps[:],
                        xgT_sb[:, kt, t * 128:(t + 1) * 128],
                        ws_sb[:, kt, :],
                        start=(kt == 0), stop=False,
                    )
                main_ps[t] = ps

            for ch in range(XCH):
                xch = xpool.tile([128, RJ, D], bf16, tag="x")
                nc.sync.dma_start(xch[:], x_d[ch].rearrange("p (j d) -> p j d", d=D))
                if ch == 0:
                    xgT_sb = cpool.tile([128, KT, ML], bf16)
                    nc.sync.dma_start(
                        xgT_sb[:], xgT_d.rearrange("p (t m) -> p t m", m=ML))
                    ws_sb = cpool.tile([128, KT, D], bf16)
                    nc.sync.dma_start(
                        ws_sb[:], ws_d.rearrange("p (t n) -> p t n", n=D))
                for j in range(RJ):
                    nc.tensor.matmul(psS[ch * SW:(ch + 1) * SW, :],
                                     g_sb[:, j, :], xch[:, j, :],
                                     start=(j == 0), stop=(j == RJ - 1),
                                     tile_position=(0, ch * SW))
                open_main_group(ch)   # fill PE while next chunk streams

            s_nat = cpool.tile([128, D], bf16)
            nc.scalar.copy(s_nat[:], psS[:])

            # ---- transpose S -> S^T (bf16) ----
            s_bf = cpool.tile([128, KT, BL], bf16)
            for dt in range(KT):
                psT = ptpool.tile([128, BL], bf16, tag="psT")
                nc.tensor.transpose(psT[:], s_nat[:, dt * 128:(dt + 1) * 128],
                                    id_sb[:])
                nc.vector.tensor_copy(s_bf[:, dt, :], psT[:])

            # ---- A = S @ W_nbr + b (bias via K=1 ones matmul) ----
            psA = papool.tile([128, D], f32)
            for kt in range(KT):
                nc.tensor.matmul(psA[:], s_bf[:, kt, :], wn_sb[:, kt, :],
                                 start=(kt == 0), stop=False)
            nc.tensor.matmul(psA[:], ones_sb[:], b_sb[:],
                             start=False, stop=True)
            a_bf = cpool.tile([128, D], bf16)
            nc.vector.tensor_copy(a_bf[:], psA[:])

            # ---- close groups / remaining tiles; stores in pairs ----
            out_r = out_d.rearrange("(t u p) n -> t p u n", p=128, u=2)
            ot = None
            for t in range(MT):
                if t not in main_ps:
                    open_main_group(t)
                ps = main_ps.pop(t)
                nc.tensor.matmul(ps[:], eT_sb[:, t * 128:(t + 1) * 128],
                                 a_bf[:], start=False, stop=True)
                if t % 2 == 0:
                    ot = opool.tile([128, 2, D], f32, tag="ot")
                nc.scalar.activation(ot[:, t % 2, :], ps[:],
                                     mybir.ActivationFunctionType.Relu)
                if t % 2 == 1:
                    nc.sync.dma_start(out_r[t // 2], ot[:])

    nc.compile()
    return nc


def _get_compiled():
    global _compiled
    if _compiled is None:
        _compiled = _build_bass()
    return _compiled


def _host_prep(inputs):
    """Shard + preprocess on host. Returns per-core input maps."""
    x = np.asarray(inputs["spatial_branch_feature_map"], dtype=np.float32)
    W_self = np.asarray(inputs["W_self"], dtype=np.float32)
    W_nbr = np.asarray(inputs["W_nbr"], dtype=np.float32)
    b = np.asarray(inputs["b"], dtype=np.float32)
    st = np.asarray(inputs["slicing_tensor"])
    op = np.asarray(inputs["object_pairs"])

    N = x.shape[0]
    n = NOBJ
    # exact replication of the reference's LUT-based row computation
    keys = st[:, 0].astype(np.int64) * (n * n) + st[:, 1].astype(np.int64) * n \
        + st[:, 2].astype(np.int64)
    lut = np.zeros(B * n * n, dtype=np.int64)
    lut[keys] = np.arange(N, dtype=np.int64)
    pmin = np.minimum(op[..., 0], op[..., 1]).astype(np.int64)
    pmax = np.maximum(op[..., 0], op[..., 1]).astype(np.int64)
    rel_keys = (np.arange(B, dtype=np.int64)[:, None] * (n * n)
                + pmin * n + pmax).reshape(-1)
    rows = lut[rel_keys]                      # [B*MAXR] global row index

    xg = x[rows]                              # [B*MAXR, D]
    # x: [NCORES, XCH, 128, RJ*D]; sbuf[p, j, :] = x_core[ch*896 + j*128 + p]
    x_bf = np.ascontiguousarray(
        x.astype(BF16).reshape(NCORES, XCH, RJ, 128, D)
        .transpose(0, 1, 3, 2, 4).reshape(NCORES, XCH, 128, RJ * D))
    # xgT: [NCORES, 128, KT*ML]; sbuf[p, kt, m] = xg_core[m, kt*128+p]
    xgT = np.ascontiguousarray(
        xg.astype(BF16).reshape(NCORES, ML, KT, 128)
        .transpose(0, 3, 2, 1).reshape(NCORES, 128, KT * ML))

    def wlay(W):  # [D, D] -> [128, KT*D]: sbuf[p, kt, n] = W[kt*128+p, n]
        return np.ascontiguousarray(
            W.astype(BF16).reshape(KT, 128, D).transpose(1, 0, 2)
            .reshape(128, KT * D))

    ws = wlay(W_self)
    wn = wlay(W_nbr)
    eT = (np.arange(ML)[None, :] // MAXR
          == np.arange(128)[:, None]).astype(BF16)   # [128, ML]
    # shared one-hot block: g[p, j*SW + s] = ((j*128 + p)//NC2 == s)
    jj = np.arange(RJ * 128)
    g = (jj[:, None] // NC2 == np.arange(SW)[None, :]).astype(BF16)
    g = np.ascontiguousarray(
        g.reshape(RJ, 128, SW).transpose(1, 0, 2).reshape(128, RJ * SW))
    bias = b.astype(BF16).reshape(1, D)
    ident = np.eye(128, dtype=BF16)

    in_maps = []
    for c in range(NCORES):
        in_maps.append({
            "x": x_bf[c], "xgT": xgT[c], "g": g,
            "ws": ws, "wn": wn, "eT": eT, "bias": bias, "ident": ident,
        })
    return in_maps


def run(inputs, trace=False):
    """Returns (full_output, BassKernelResults)."""
    from concourse.bass_utils import run_bass_kernel_spmd

    nc = _get_compiled()
    in_maps = _host_prep(inputs)
    res = run_bass_kernel_spmd(nc, in_maps, core_ids=list(range(NCORES)),
                               trace=trace)
    out = np.concatenate([r["out"] for r in res.results], axis=0)
    return out, res


def kernel(**inputs) -> np.ndarray:
    out, _ = run(inputs, trace=False)
    return out



# revision 9
# speedup vs baseline: 1.0043x; 1.0043x over previous
"""Trainium2 Bass kernel for the GraphicalBranch GNN message-passing problem.

Math (equivalent to the reference):
  - Per-sample graphs are fully connected WITH self-loops over the nc2=28
    pair-nodes, so segment_sum(x[src], dst) == broadcast of the per-sample
    row-sum S[b] = sum_r x[b, r, :].
  - The final key-matching gather h[rows] commutes with the row-wise linear
    layer, so only the 10 gathered rows per sample are pushed through W_self:
        out[b*10+k] = relu(xg[b*10+k] @ W_self + (S[b] @ W_nbr + b))
  - rows computed on host from slicing_tensor/object_pairs (index arithmetic).

Sharding: data-parallel over samples; each of 8 cores gets 128 samples
(3584 x-rows, 1280 output rows). Weights replicated.

Per-core schedule (matmul operands bf16, f32 PSUM):
  - x streams in 4 chunks of 896 rows (=32 samples). Chunk c's 7 G-matmuls
    accumulate S rows [32c,32c+32) into one PSUM bank (tile_position col 32c).
  - A = S @ W_nbr + b is computed in TWO halves: samples 0-63 right after
    chunk 1, samples 64-127 after chunk 3 (copy psS half -> sbuf bf16, 4 PE
    transposes of 64 cols -> S^T, 4 k-matmuls M=64 into psA[0:64]).  Output
    tile t only references samples [12.8t, 12.8t+12.7], so tiles 0-4 close
    with half 0 while x chunks 2-3 still stream; the post-x3 tail is just
    chunk3's S-mats + A-half1 + 5 closes.
  - W_self groups: per out tile t, 4 matmuls lhsT=xgT[t] slices, rhs=ws.
    Each group is closed by ONE expansion matmul with K=65: lhsT = per-half
    one-hot eTh (64 sample rows + a ones row), rhs = abf (A rows + bias row),
    which adds E@A AND the bias in a single pass.
  - ReLU on ScalarE psum->sbuf bf16; paired stores on the sync ring after all
    load issues. Host upcasts bf16 -> f32.

Load order: sync ring x0, ws, xgT[tiles 0-4], x1, x2, xgT[5-9], x3 (each load
has its own preallocated buffer -- no issue-side waits); scalar ring g, ident,
eTh, bias rows, wn. PSUM: psS + psT + psA + 5 live out-groups = 8 banks.
"""

import numpy as np
import ml_dtypes

# ---- problem constants (hardcoded; kernel.py must be self-contained) ----
B = 1024          # samples
NOBJ = 8          # objects per sample
NC2 = 28          # pair-nodes per sample
MAXR = 10         # relations per sample
D = 512           # feature dim
NCORES = 8
BL = B // NCORES          # 128 samples per core
RL = BL * NC2             # 3584 x-rows per core
ML = BL * MAXR            # 1280 output rows per core
KT = D // 128             # 4 contraction tiles
MT = ML // 128            # 10 output row tiles per core
XCH = 4                   # x chunks (896 rows = 32 samples each)
RJ = 7                    # row-tiles per chunk
SW = BL // XCH            # 32 samples per chunk
HM = ML // 2              # 640 output rows per half

BF16 = ml_dtypes.bfloat16

_compiled = None


def _build_bass():
    import concourse.bacc as bacc
    import concourse.bass as bass
    import concourse.mybir as mybir
    from concourse import tile

    f32 = mybir.dt.float32
    bf16 = mybir.dt.bfloat16

    nc = bacc.Bacc("TRN2", target_bir_lowering=False, debug=False,
                   num_devices=NCORES)

    # host-prelaid, partition-major contiguous inputs
    x_d = nc.dram_tensor("x", [XCH, 128, RJ * D], bf16, kind="ExternalInput")
    g_d = nc.dram_tensor("g", [128, RJ * SW], bf16, kind="ExternalInput")
    # xgT m-major: [128, tile t, kt, 128] so a tile range is contiguous
    xgT_d = nc.dram_tensor("xgT", [128, MT * KT * 128], bf16,
                           kind="ExternalInput")
    ws_d = nc.dram_tensor("ws", [128, KT * D], bf16, kind="ExternalInput")
    wn_d = nc.dram_tensor("wn", [128, KT * D], bf16, kind="ExternalInput")
    # per-half expansion blocks: 64 one-hot sample rows + ones row (bias)
    eT_d = nc.dram_tensor("eTh", [2, 65, HM], bf16, kind="ExternalInput")
    b_d = nc.dram_tensor("bias", [1, D], bf16, kind="ExternalInput")
    id_d = nc.dram_tensor("ident", [128, 128], bf16, kind="ExternalInput")
    out_d = nc.dram_tensor("out", [ML, D], bf16, kind="ExternalOutput")

    with tile.TileContext(nc) as tc:
        with (
            tc.tile_pool(name="const", bufs=1) as cpool,
            tc.tile_pool(name="outp", bufs=3) as opool,
            tc.tile_pool(name="psum", bufs=5, space=bass.MemorySpace.PSUM) as ppool,
            tc.tile_pool(name="psumS", bufs=1, space=bass.MemorySpace.PSUM) as pspool,
            tc.tile_pool(name="psumT", bufs=1, space=bass.MemorySpace.PSUM) as ptpool,
            tc.tile_pool(name="psumA", bufs=1, space=bass.MemorySpace.PSUM) as papool,
        ):
            # abf_h: rows 0-63 = A for samples [64h, 64h+64), row 64 = bias
            abf = [cpool.tile([65, D], bf16, name=f"abf{h}", tag=f"abf{h}")
                   for h in range(2)]

            # ---- scalar-ring loads (small, consumed throughout) ----
            g_sb = cpool.tile([128, RJ, SW], bf16)
            nc.scalar.dma_start(g_sb[:], g_d.rearrange("p (j s) -> p j s", s=SW))
            id_sb = cpool.tile([128, 128], bf16)
            nc.scalar.dma_start(id_sb[:], id_d[:, :])
            eT_sb = [cpool.tile([65, HM], bf16, name=f"eT{h}", tag=f"eT{h}")
                     for h in range(2)]
            nc.scalar.dma_start(eT_sb[0][:], eT_d[0])
            nc.scalar.dma_start(eT_sb[1][:], eT_d[1])
            nc.scalar.dma_start(abf[0][64:65, :], b_d[:, :])
            nc.scalar.dma_start(abf[1][64:65, :], b_d[:, :])
            wn_sb = cpool.tile([128, KT, D], bf16)
            nc.scalar.dma_start(wn_sb[:], wn_d.rearrange("p (t n) -> p t n", n=D))

            # ---- sync-ring loads: x0, ws, xgT05, x1, x2, xgT59, x3 ----
            xgT_r = xgT_d.rearrange("p (t k m) -> p t k m", k=KT, m=128)
            xch = [cpool.tile([128, RJ, D], bf16, name=f"x{c}", tag=f"x{c}")
                   for c in range(XCH)]
            xgT_sb = cpool.tile([128, MT, KT, 128], bf16)
            ws_sb = cpool.tile([128, KT, D], bf16)

            nc.sync.dma_start(xch[0][:], x_d[0].rearrange("p (j d) -> p j d", d=D))
            nc.sync.dma_start(ws_sb[:], ws_d.rearrange("p (t n) -> p t n", n=D))
            nc.sync.dma_start(xgT_sb[:, 0:5], xgT_r[:, 0:5])
            nc.sync.dma_start(xch[1][:], x_d[1].rearrange("p (j d) -> p j d", d=D))
            nc.sync.dma_start(xch[2][:], x_d[2].rearrange("p (j d) -> p j d", d=D))
            nc.sync.dma_start(xgT_sb[:, 5:10], xgT_r[:, 5:10])
            nc.sync.dma_start(xch[3][:], x_d[3].rearrange("p (j d) -> p j d", d=D))

            # ---- compute ----
            psS = pspool.tile([128, D], f32)
            s_bf = cpool.tile([128, KT, BL], bf16)     # S^T (lhsT for A)
            snat = cpool.tile([128, D], bf16)          # psS copy staging
            psA = papool.tile([128, D], f32)
            out_r = out_d.rearrange("(t u p) n -> t p u n", p=128, u=2)

            def s_chunk(c):
                for j in range(RJ):
                    nc.tensor.matmul(psS[c * SW:(c + 1) * SW, :],
                                     g_sb[:, j, :], xch[c][:, j, :],
                                     start=(j == 0), stop=(j == RJ - 1),
                                     tile_position=(0, c * SW))

            def a_half(h):
                lo, hi = 64 * h, 64 * h + 64
                nc.scalar.copy(snat[lo:hi, :], psS[lo:hi, :])
                for dt in range(KT):
                    psT = ptpool.tile([128, 64], bf16, tag="psT")
                    nc.tensor.transpose(psT[:],
                                        snat[lo:hi, dt * 128:(dt + 1) * 128],
                                        id_sb[lo:hi, lo:hi])
                    nc.vector.tensor_copy(s_bf[:, dt, lo:hi], psT[:])
                # A rows for this half -> psA[0:64] (partitions 0-63 so the
                # bf16 copy to abf[h][0:64] stays partition-aligned)
                for kt in range(KT):
                    nc.tensor.matmul(psA[0:64, :], s_bf[:, kt, lo:hi],
                                     wn_sb[:, kt, :],
                                     start=(kt == 0), stop=(kt == KT - 1),
                                     tile_position=(0, 0))
                nc.vector.tensor_copy(abf[h][0:64, :], psA[0:64, :])

            main_ps = {}

            def open_group(t):
                ps = ppool.tile([128, D], f32, tag="ps")
                for kt in range(KT):
                    nc.tensor.matmul(ps[:], xgT_sb[:, t, kt, :], ws_sb[:, kt, :],
                                     start=(kt == 0), stop=False)
                main_ps[t] = ps

            ot = None

            def close_group(t):
                h, tl = t // 5, t % 5
                ps = main_ps.pop(t)
                nc.tensor.matmul(ps[:], eT_sb[h][:, tl * 128:(tl + 1) * 128],
                                 abf[h][:, :], start=False, stop=True)
                nonlocal ot
                if t % 2 == 0:
                    ot = opool.tile([128, 2, D], bf16, tag="ot")
                nc.scalar.activation(ot[:, t % 2, :], ps[:],
                                     mybir.ActivationFunctionType.Relu)
                if t % 2 == 1:
                    nc.sync.dma_start(out_r[t // 2], ot[:])

            s_chunk(0)              # fills the PE while ws/xgT stream
            for t in range(5):
                open_group(t)
            s_chunk(1)
            a_half(0)               # A for samples 0-63
            for t in range(5):      # tiles 0-4 only touch samples 0-63
                close_group(t)
            s_chunk(2)
            for t in range(5, 10):
                open_group(t)
            s_chunk(3)
            a_half(1)               # A for samples 64-127
            for t in range(5, 10):
                close_group(t)

    nc.compile()
    return nc


def _get_compiled():
    global _compiled
    if _compiled is None:
        _compiled = _build_bass()
    return _compiled


def _host_prep(inputs):
    """Shard + preprocess on host. Returns per-core input maps."""
    x = np.asarray(inputs["spatial_branch_feature_map"], dtype=np.float32)
    W_self = np.asarray(inputs["W_self"], dtype=np.float32)
    W_nbr = np.asarray(inputs["W_nbr"], dtype=np.float32)
    b = np.asarray(inputs["b"], dtype=np.float32)
    st = np.asarray(inputs["slicing_tensor"])
    op = np.asarray(inputs["object_pairs"])

    N = x.shape[0]
    n = NOBJ
    # exact replication of the reference's LUT-based row computation
    keys = st[:, 0].astype(np.int64) * (n * n) + st[:, 1].astype(np.int64) * n \
        + st[:, 2].astype(np.int64)
    lut = np.zeros(B * n * n, dtype=np.int64)
    lut[keys] = np.arange(N, dtype=np.int64)
    pmin = np.minimum(op[..., 0], op[..., 1]).astype(np.int64)
    pmax = np.maximum(op[..., 0], op[..., 1]).astype(np.int64)
    rel_keys = (np.arange(B, dtype=np.int64)[:, None] * (n * n)
                + pmin * n + pmax).reshape(-1)
    rows = lut[rel_keys]                      # [B*MAXR] global row index

    xg = x[rows]                              # [B*MAXR, D]
    # x: [NCORES, XCH, 128, RJ*D]; sbuf[p, j, :] = x_core[ch*896 + j*128 + p]
    x_bf = np.ascontiguousarray(
        x.astype(BF16).reshape(NCORES, XCH, RJ, 128, D)
        .transpose(0, 1, 3, 2, 4).reshape(NCORES, XCH, 128, RJ * D))
    # xgT m-major: sbuf[p, t, kt, m] = xg_core[t*128 + m, kt*128 + p]
    xgT = np.ascontiguousarray(
        xg.astype(BF16).reshape(NCORES, MT, 128, KT, 128)
        .transpose(0, 4, 1, 3, 2).reshape(NCORES, 128, MT * KT * 128))

    def wlay(W):  # [D, D] -> [128, KT*D]: sbuf[p, kt, n] = W[kt*128+p, n]
        return np.ascontiguousarray(
            W.astype(BF16).reshape(KT, 128, D).transpose(1, 0, 2)
            .reshape(128, KT * D))

    ws = wlay(W_self)
    wn = wlay(W_nbr)
    # eTh[h, i<64, m] = ((640h + m)//10 == 64h + i); eTh[h, 64, m] = 1 (bias)
    eTh = np.zeros((2, 65, HM), dtype=BF16)
    for h in range(2):
        m = np.arange(HM) + h * HM
        eTh[h, :64] = ((m[None, :] // MAXR)
                       == (np.arange(64)[:, None] + 64 * h)).astype(BF16)
        eTh[h, 64] = BF16(1.0)
    # shared one-hot block: g[p, j*SW + s] = ((j*128 + p)//NC2 == s)
    jj = np.arange(RJ * 128)
    g = (jj[:, None] // NC2 == np.arange(SW)[None, :]).astype(BF16)
    g = np.ascontiguousarray(
        g.reshape(RJ, 128, SW).transpose(1, 0, 2).reshape(128, RJ * SW))
    bias = b.astype(BF16).reshape(1, D)
    ident = np.eye(128, dtype=BF16)

    in_maps = []
    for c in range(NCORES):
        in_maps.append({
            "x": x_bf[c], "xgT": xgT[c], "g": g,
            "ws": ws, "wn": wn, "eTh": eTh, "bias": bias, "ident": ident,
        })
    return in_maps


def run(inputs, trace=False):
    """Returns (full_output, BassKernelResults)."""
    from concourse.bass_utils import run_bass_kernel_spmd

    nc = _get_compiled()
    in_maps = _host_prep(inputs)
    res = run_bass_kernel_spmd(nc, in_maps, core_ids=list(range(NCORES)),
                               trace=trace)
    out = np.concatenate([r["out"] for r in res.results], axis=0)
    return out.astype(np.float32), res


def kernel(**inputs) -> np.ndarray:
    out, _ = run(inputs, trace=False)
    return out


# revision 10
# speedup vs baseline: 1.1129x; 1.1081x over previous
"""Trainium2 Bass kernel for the GraphicalBranch GNN message-passing problem.

Math (equivalent to the reference):
  - Per-sample graphs are fully connected WITH self-loops over the nc2=28
    pair-nodes, so segment_sum(x[src], dst) == broadcast of the per-sample
    row-sum S[b] = sum_r x[b, r, :].
  - The final key-matching gather h[rows] commutes with the row-wise linear
    layer, so only the 10 gathered rows per sample are pushed through W_self:
        out[b*10+k] = relu(xg[b*10+k] @ W_self + (S[b] @ W_nbr + b))
  - rows computed on host from slicing_tensor/object_pairs (index arithmetic).

Sharding: data-parallel over samples; each of 8 cores gets 128 samples
(3584 x-rows, 1280 output rows). Weights replicated.

Per-core schedule:
  - x (bf16) streams in 4 chunks of 896 rows (=32 samples); chunk 0 is split
    so the PE starts after just 256 rows.  Chunk c's 7 G-matmuls accumulate
    S rows [32c,32c+32) into one PSUM bank (tile_position col 32c).
  - xg @ W_self runs in fp8 e4m3 with perf_mode=DoubleRow (2 k-subtiles per
    matmul, 2 rows/cycle): per out tile t, 2 DR-matmuls lhsT=xgT8[t] k-pairs,
    rhs=ws8 k-pairs.  The S/A path stays bf16 (fp8 there fails the 2e-2 gate).
  - A = S @ W_nbr + b in TWO halves: samples 0-63 after chunk 1, 64-127 after
    chunk 3 (psS half -> sbuf bf16, 4 PE transposes, 4 k-matmuls M=64 into
    psA[0:64]).  Output tile t only references samples [12.8t, 12.8t+12.7],
    so tiles 0-4 close with half 0 while x chunks 2-3 still stream.
  - Each W_self group closes with ONE K=65 expansion matmul: lhsT = per-half
    one-hot eTh (64 sample rows + ones row), rhs = abf (A rows + bias row),
    adding E@A AND the bias in a single pass.
  - ReLU on ScalarE psum->sbuf bf16; paired stores on the sync ring after all
    load issues.  Host upcasts bf16 -> f32.

Emission order keeps the PE busy continuously (p-state ramps to 2.4 GHz after
3us of uninterrupted work): s0a s0b groups0-4 s1 s2 [snat0 on ScalarE under
s2] transp-h0 A0 closes0-4 s3 groups5-9 [snat1 under groups] transp-h1 A1
closes5-9.  PSUM: psS + psT + psA + 5 live out-groups = 8 banks.
"""

import numpy as np
import ml_dtypes

# ---- problem constants (hardcoded; kernel.py must be self-contained) ----
B = 1024          # samples
NOBJ = 8          # objects per sample
NC2 = 28          # pair-nodes per sample
MAXR = 10         # relations per sample
D = 512           # feature dim
NCORES = 8
BL = B // NCORES          # 128 samples per core
RL = BL * NC2             # 3584 x-rows per core
ML = BL * MAXR            # 1280 output rows per core
KT = D // 128             # 4 contraction tiles
MT = ML // 128            # 10 output row tiles per core
XCH = 4                   # x chunks (896 rows = 32 samples each)
RJ = 7                    # row-tiles per chunk
SW = BL // XCH            # 32 samples per chunk
HM = ML // 2              # 640 output rows per half

BF16 = ml_dtypes.bfloat16
FP8 = ml_dtypes.float8_e4m3

_compiled = None


def _build_bass():
    import concourse.bacc as bacc
    import concourse.bass as bass
    import concourse.mybir as mybir
    from concourse import tile

    f32 = mybir.dt.float32
    bf16 = mybir.dt.bfloat16
    fp8 = mybir.dt.float8e4
    DR = mybir.MatmulPerfMode.DoubleRow

    nc = bacc.Bacc("TRN2", target_bir_lowering=False, debug=False,
                   num_devices=NCORES)

    # host-prelaid, partition-major contiguous inputs
    x_d = nc.dram_tensor("x", [XCH, 128, RJ * D], bf16, kind="ExternalInput")
    g_d = nc.dram_tensor("g", [128, RJ * SW], bf16, kind="ExternalInput")
    # xgT m-major: [128, tile t, kt, 128] so a tile range is contiguous
    xgT_d = nc.dram_tensor("xgT", [128, MT * KT * 128], fp8,
                           kind="ExternalInput")
    ws_d = nc.dram_tensor("ws", [128, KT * D], fp8, kind="ExternalInput")
    wn_d = nc.dram_tensor("wn", [128, KT * D], bf16, kind="ExternalInput")
    # per-half expansion blocks: 64 one-hot sample rows + ones row (bias)
    eT_d = nc.dram_tensor("eTh", [2, 65, HM], bf16, kind="ExternalInput")
    b_d = nc.dram_tensor("bias", [1, D], bf16, kind="ExternalInput")
    id_d = nc.dram_tensor("ident", [128, 128], bf16, kind="ExternalInput")
    out_d = nc.dram_tensor("out", [ML, D], bf16, kind="ExternalOutput")

    with tile.TileContext(nc) as tc:
        with (
            tc.tile_pool(name="const", bufs=1) as cpool,
            tc.tile_pool(name="outp", bufs=3) as opool,
            tc.tile_pool(name="psum", bufs=5, space=bass.MemorySpace.PSUM) as ppool,
            tc.tile_pool(name="psumS", bufs=1, space=bass.MemorySpace.PSUM) as pspool,
            tc.tile_pool(name="psumT", bufs=1, space=bass.MemorySpace.PSUM) as ptpool,
            tc.tile_pool(name="psumA", bufs=1, space=bass.MemorySpace.PSUM) as papool,
        ):
            # abf_h: rows 0-63 = A for samples [64h, 64h+64), row 64 = bias
            abf = [cpool.tile([65, D], bf16, name=f"abf{h}", tag=f"abf{h}")
                   for h in range(2)]

            # ---- scalar-ring loads (small, consumed throughout) ----
            g_sb = cpool.tile([128, RJ, SW], bf16)
            nc.scalar.dma_start(g_sb[:], g_d.rearrange("p (j s) -> p j s", s=SW))
            id_sb = cpool.tile([128, 128], bf16)
            nc.scalar.dma_start(id_sb[:], id_d[:, :])
            eT_sb = [cpool.tile([65, HM], bf16, name=f"eT{h}", tag=f"eT{h}")
                     for h in range(2)]
            nc.scalar.dma_start(eT_sb[0][:], eT_d[0])
            nc.scalar.dma_start(eT_sb[1][:], eT_d[1])
            nc.scalar.dma_start(abf[0][64:65, :], b_d[:, :])
            nc.scalar.dma_start(abf[1][64:65, :], b_d[:, :])
            wn_sb = cpool.tile([128, KT, D], bf16)
            nc.scalar.dma_start(wn_sb[:], wn_d.rearrange("p (t n) -> p t n", n=D))

            # ---- sync-ring loads: x0a, x0b, ws, xgT05, x1, x2, xgT59, x3 ----
            xgT_r = xgT_d.rearrange("p (t k m) -> p t k m", k=KT, m=128)
            xch = [cpool.tile([128, RJ, D], bf16, name=f"x{c}", tag=f"x{c}")
                   for c in range(XCH)]
            xgT_sb = cpool.tile([128, MT, KT, 128], fp8)
            ws_sb = cpool.tile([128, KT, D], fp8)
            x_r = [x_d[c].rearrange("p (j d) -> p j d", d=D) for c in range(XCH)]

            nc.sync.dma_start(xch[0][:, 0:2], x_r[0][:, 0:2])
            nc.sync.dma_start(xch[0][:, 2:RJ], x_r[0][:, 2:RJ])
            nc.sync.dma_start(ws_sb[:], ws_d.rearrange("p (t n) -> p t n", n=D))
            nc.sync.dma_start(xgT_sb[:, 0:5], xgT_r[:, 0:5])
            nc.sync.dma_start(xch[1][:], x_r[1])
            nc.sync.dma_start(xch[2][:], x_r[2])
            nc.sync.dma_start(xgT_sb[:, 5:10], xgT_r[:, 5:10])
            nc.sync.dma_start(xch[3][:], x_r[3])

            # ---- compute ----
            psS = pspool.tile([128, D], f32)
            s_bf = cpool.tile([128, KT, BL], bf16)     # S^T (lhsT for A)
            snat = cpool.tile([128, D], bf16)          # psS copy staging
            psA = papool.tile([128, D], f32)
            out_r = out_d.rearrange("(t u p) n -> t p u n", p=128, u=2)

            def s_chunk(c):
                for j in range(RJ):
                    nc.tensor.matmul(psS[c * SW:(c + 1) * SW, :],
                                     g_sb[:, j, :], xch[c][:, j, :],
                                     start=(j == 0), stop=(j == RJ - 1),
                                     tile_position=(0, c * SW))

            def snat_copy(h):
                lo, hi = 64 * h, 64 * h + 64
                nc.scalar.copy(snat[lo:hi, :], psS[lo:hi, :])

            def a_half(h):
                lo, hi = 64 * h, 64 * h + 64
                for dt in range(KT):
                    psT = ptpool.tile([128, 64], bf16, tag="psT")
                    nc.tensor.transpose(psT[:],
                                        snat[lo:hi, dt * 128:(dt + 1) * 128],
                                        id_sb[lo:hi, lo:hi])
                    nc.vector.tensor_copy(s_bf[:, dt, lo:hi], psT[:])
                # A rows for this half -> psA[0:64] (partitions 0-63 so the
                # bf16 copy to abf[h][0:64] stays partition-aligned)
                for kt in range(KT):
                    nc.tensor.matmul(psA[0:64, :], s_bf[:, kt, lo:hi],
                                     wn_sb[:, kt, :],
                                     start=(kt == 0), stop=(kt == KT - 1),
                                     tile_position=(0, 0))
                nc.vector.tensor_copy(abf[h][0:64, :], psA[0:64, :])

            main_ps = {}

            def open_group(t):
                ps = ppool.tile([128, D], f32, tag="ps")
                for kp in range(KT // 2):
                    nc.tensor.matmul(ps[:], xgT_sb[:, t, 2 * kp:2 * kp + 2, :],
                                     ws_sb[:, 2 * kp:2 * kp + 2, :],
                                     start=(kp == 0), stop=False,
                                     perf_mode=DR)
                main_ps[t] = ps

            ot = None

            def close_group(t):
                h, tl = t // 5, t % 5
                ps = main_ps.pop(t)
                nc.tensor.matmul(ps[:], eT_sb[h][:, tl * 128:(tl + 1) * 128],
                                 abf[h][:, :], start=False, stop=True)
                nonlocal ot
                if t % 2 == 0:
                    ot = opool.tile([128, 2, D], bf16, tag="ot")
                nc.scalar.activation(ot[:, t % 2, :], ps[:],
                                     mybir.ActivationFunctionType.Relu)
                if t % 2 == 1:
                    nc.sync.dma_start(out_r[t // 2], ot[:])

            s_chunk(0)              # fills the PE while ws/xgT stream
            for t in range(5):
                open_group(t)
            s_chunk(1)
            s_chunk(2)              # covers the snat0 ScalarE copy latency
            snat_copy(0)
            a_half(0)               # A for samples 0-63
            for t in range(5):      # tiles 0-4 only touch samples 0-63
                close_group(t)
            s_chunk(3)
            snat_copy(1)            # runs under groups 5-9
            for t in range(5, 10):
                open_group(t)
            a_half(1)               # A for samples 64-127
            for t in range(5, 10):
                close_group(t)

    nc.compile()
    return nc


def _get_compiled():
    global _compiled
    if _compiled is None:
        _compiled = _build_bass()
    return _compiled


def _host_prep(inputs):
    """Shard + preprocess on host. Returns per-core input maps."""
    x = np.asarray(inputs["spatial_branch_feature_map"], dtype=np.float32)
    W_self = np.asarray(inputs["W_self"], dtype=np.float32)
    W_nbr = np.asarray(inputs["W_nbr"], dtype=np.float32)
    b = np.asarray(inputs["b"], dtype=np.float32)
    st = np.asarray(inputs["slicing_tensor"])
    op = np.asarray(inputs["object_pairs"])

    N = x.shape[0]
    n = NOBJ
    # exact replication of the reference's LUT-based row computation
    keys = st[:, 0].astype(np.int64) * (n * n) + st[:, 1].astype(np.int64) * n \
        + st[:, 2].astype(np.int64)
    lut = np.zeros(B * n * n, dtype=np.int64)
    lut[keys] = np.arange(N, dtype=np.int64)
    pmin = np.minimum(op[..., 0], op[..., 1]).astype(np.int64)
    pmax = np.maximum(op[..., 0], op[..., 1]).astype(np.int64)
    rel_keys = (np.arange(B, dtype=np.int64)[:, None] * (n * n)
                + pmin * n + pmax).reshape(-1)
    rows = lut[rel_keys]                      # [B*MAXR] global row index

    xg = x[rows]                              # [B*MAXR, D]
    # x: [NCORES, XCH, 128, RJ*D]; sbuf[p, j, :] = x_core[ch*896 + j*128 + p]
    x_bf = np.ascontiguousarray(
        x.astype(BF16).reshape(NCORES, XCH, RJ, 128, D)
        .transpose(0, 1, 3, 2, 4).reshape(NCORES, XCH, 128, RJ * D))
    # xgT m-major fp8: sbuf[p, t, kt, m] = xg_core[t*128 + m, kt*128 + p]
    xgT = np.ascontiguousarray(
        xg.astype(FP8).reshape(NCORES, MT, 128, KT, 128)
        .transpose(0, 4, 1, 3, 2).reshape(NCORES, 128, MT * KT * 128))

    def wlay(W, dt):  # [D, D] -> [128, KT*D]: sbuf[p, kt, n] = W[kt*128+p, n]
        return np.ascontiguousarray(
            W.astype(dt).reshape(KT, 128, D).transpose(1, 0, 2)
            .reshape(128, KT * D))

    ws = wlay(W_self, FP8)
    wn = wlay(W_nbr, BF16)
    # eTh[h, i<64, m] = ((640h + m)//10 == 64h + i); eTh[h, 64, m] = 1 (bias)
    eTh = np.zeros((2, 65, HM), dtype=BF16)
    for h in range(2):
        m = np.arange(HM) + h * HM
        eTh[h, :64] = ((m[None, :] // MAXR)
                       == (np.arange(64)[:, None] + 64 * h)).astype(BF16)
        eTh[h, 64] = BF16(1.0)
    # shared one-hot block: g[p, j*SW + s] = ((j*128 + p)//NC2 == s)
    jj = np.arange(RJ * 128)
    g = (jj[:, None] // NC2 == np.arange(SW)[None, :]).astype(BF16)
    g = np.ascontiguousarray(
        g.reshape(RJ, 128, SW).transpose(1, 0, 2).reshape(128, RJ * SW))
    bias = b.astype(BF16).reshape(1, D)
    ident = np.eye(128, dtype=BF16)

    in_maps = []
    for c in range(NCORES):
        in_maps.append({
            "x": x_bf[c], "xgT": xgT[c], "g": g,
            "ws": ws, "wn": wn, "eTh": eTh, "bias": bias, "ident": ident,
        })
    return in_maps


def run(inputs, trace=False):
    """Returns (full_output, BassKernelResults)."""
    from concourse.bass_utils import run_bass_kernel_spmd

    nc = _get_compiled()
    in_maps = _host_prep(inputs)
    res = run_bass_kernel_spmd(nc, in_maps, core_ids=list(range(NCORES)),
                               trace=trace)
    out = np.concatenate([r["out"] for r in res.results], axis=0)
    return out.astype(np.float32), res


def kernel(**inputs) -> np.ndarray:
    out, _ = run(inputs, trace=False)
    return out
